# revision 67
# baseline (speedup 1.0000x reference)
"""Fused LayerNorm + multi-head attention + output projection on 8 TRN2 NeuronCores.

Sharding: 2-way data parallel over batch x 4-way tensor parallel over heads.
Core c handles batch (c // 4), heads [4*(c%4) .. 4*(c%4)+4).

Device dataflow (everything transposed: host supplies x^T so the feature/
contraction dim always lands on SBUF partitions):
  - LayerNorm is folded into the QKV-projection epilogue:
      qkv^T[n,i] = rstd_i * (raw[n,i] - mu_i * wsum_n) (+ wb_n)
    with raw = W'^T x^T computed on raw x, row stats (mu, rstd) from
    PE ones-matmuls (which broadcast across partitions for free).
  - Scores are computed transposed (S^T[j,i]) so softmax'd probs feed the
    PV matmul without any transpose; two 64-dim heads are packed into the
    128 PE rows via row groups.
  - Softmax skips max-subtraction (values are bounded; a constant bias in
    the exp cancels in the normalization). The denominator comes from an
    extra ones-column appended to V (M=65 PV matmuls); normalization
    reads the PV PSUM in place on DVE (no ACT copies) with only the
    l-rows going through a partition-0 DMA + reciprocal + gpsimd
    broadcast.
  - i-chunk-major loop: for each 512-token chunk both head pairs run
    attention, then the chunk's output projection is dripped into the
    next chunk's attention loop, so out-proj/PE work and output DMA are
    spread across the whole timeline. The last chunk's projection is
    split: the pair-0 half runs during the final attention loop, only
    the pair-1 half + combine remain in the tail.
  - A seeded drip queue pre-emits stats / k,v groups / v-transposes with
    a multi-iteration lead so their DVE epilogues never stall the PE,
    and paces leftovers into the late, exp-bound iterations.
  - All inputs are host-packed into [128, *] slabs (one DMA each; SP
    dispatch is ~0.7us per DMA): x as 4+1 i-chunk slabs, wqkv split so
    the 0.5MB needed by the first iteration (q01+k01) lands first.
  - x^2 for the ic0 variance stats alternates DVE/GPSIMD (halves the
    head-critical chain); later chunks use idle GPSIMD only.
  - Output projection produces partial^T per core; host sums the 4 TP
    partials per batch, adds b_out, and transposes back.
"""

import os
import sys
from collections import deque

import numpy as np

for _p in ("/root/.axon_site", "/root/.axon_site/_ro/trn_rl_repo",
           "/root/.axon_site/_ro/pypackages", "/opt/trn_rl_repo"):
    if os.path.isdir(_p) and _p not in sys.path:
        sys.path.append(_p)

B = 2
N = 2048
DIM = 1024
HEADS = 16
DIM_HEAD = 64
INNER = HEADS * DIM_HEAD
HEADS_PER_CORE = 4          # 4-way tensor parallel on heads
N_CORES = 8
EPS = 1e-5
EXP_BIAS = -4.0             # constant subtracted inside exp; cancels in softmax

KT = DIM // 128             # 8 k-tiles of the contraction dim
IC = 4                      # i-chunks of 512 over N=2048
ICW = N // IC               # 512
JT = N // 128               # 16 j-tiles
NQKV = 3 * HEADS_PER_CORE * DIM_HEAD   # 768 local qkv columns
NT = NQKV // 128            # 6 n-tiles: [q01, q23, k01, k23, v01, v23]
MT = DIM // 128             # 8 output m-tiles

# engine-placement switch (GPSIMD cannot read PSUM, so only SBUF-only ops
# can move there)
GP_X2 = True                # x^2 for stats on GPSIMD instead of DVE

_COMPILED = {}


def _build(has_wb):
    import concourse.bass as bass
    import concourse.mybir as mybir
    from concourse import bacc, tile
    from concourse.masks import make_identity
    from contextlib import ExitStack

    f32 = mybir.dt.float32
    bf16 = mybir.dt.bfloat16
    AF = mybir.ActivationFunctionType
    ALU = mybir.AluOpType

    nc = bacc.Bacc("TRN2", target_bir_lowering=False, debug=False,
                   num_devices=N_CORES)

    # all inputs host-packed to 128 partitions x contiguous slabs so each
    # needs a single large DMA (dispatch on the SP queue is ~0.7us each).
    # qkv column order is [q01, k01, q23, k23, v01, v23]; wqkv is split so
    # the 0.5MB the first attention iteration needs (q01+k01) lands first.
    xT_d = nc.dram_tensor("xT", [128, IC * KT * ICW], bf16,
                          kind="ExternalInput")
    wqkA_d = nc.dram_tensor("wqkA", [128, KT * 256], bf16,
                            kind="ExternalInput")
    wqkB_d = nc.dram_tensor("wqkB", [128, KT * 512], bf16,
                            kind="ExternalInput")
    wout_d = nc.dram_tensor("wout", [128, 2 * DIM], bf16,
                            kind="ExternalInput")
    wsum_d = nc.dram_tensor("wsum", [128, NT], f32, kind="ExternalInput")
    wb_d = nc.dram_tensor("wb", [128, NT], f32, kind="ExternalInput")
    out_d = nc.dram_tensor("outT", [DIM, N], bf16, kind="ExternalOutput")

    with ExitStack() as ctx:
        tc = ctx.enter_context(tile.TileContext(nc))
        cst = ctx.enter_context(tc.tile_pool(name="cst", bufs=1))
        xp = ctx.enter_context(tc.tile_pool(name="xp", bufs=KT))
        wp = ctx.enter_context(tc.tile_pool(name="wp", bufs=KT))
        qkp = ctx.enter_context(tc.tile_pool(name="qk", bufs=1))
        vtp = ctx.enter_context(tc.tile_pool(name="vt", bufs=1))
        vaugp = ctx.enter_context(tc.tile_pool(name="vaug", bufs=JT))
        bcp = ctx.enter_context(tc.tile_pool(name="bc", bufs=1))
        scp = ctx.enter_context(tc.tile_pool(name="sc", bufs=2))
        ep = ctx.enter_context(tc.tile_pool(name="ep", bufs=8))
        onp = ctx.enter_context(tc.tile_pool(name="on", bufs=2 * IC))
        otp = ctx.enter_context(tc.tile_pool(name="ot", bufs=8))
        smp = ctx.enter_context(tc.tile_pool(name="sm", bufs=2))
        # single PSUM pool, 8 banks total:
        #   tag "s": 2 x [128,1024] (2 banks each) -> 4 banks (score tiles)
        #   tag "o": 2 x [128,512]                 -> 2 banks (PV accums)
        #   tag "g": 2 x [128,512]                 -> 2 banks
        #            (LN stats, qkv groups, v transposes, out projection)
        psum = ctx.enter_context(tc.tile_pool(name="psum", bufs=2,
                                              space="PSUM"))

        # ---- constants & weight loads ----
        ones = cst.tile([128, 128], bf16)
        nc.vector.memset(ones[:], 1.0)
        eps_t = cst.tile([128, 1], f32, tag="eps")
        nc.vector.memset(eps_t[:], EPS)
        magic_t = cst.tile([128, ICW], mybir.dt.int32, tag="magic")
        nc.vector.memset(magic_t[:], 0x5F3759DF)
        ident = cst.tile([128, 128], bf16)
        make_identity(nc, ident[:])
        wsum_t = cst.tile([128, NT], f32)
        wb_t = cst.tile([128, NT], f32)
        nc.sync.dma_start(wsum_t[:], wsum_d[:, :])
        if has_wb:
            nc.sync.dma_start(wb_t[:], wb_d[:, :])

        # x arrives host-repacked as [128, IC, KT, 512] slabs: one DMA per
        # i-chunk (the ic0 slab split in two tiles so the first stats
        # matmuls wait on a 0.5MB transfer, not the whole input load).
        xsl0 = [xp.tile([128, KT // 2 * ICW], bf16, tag=f"xs0{h}",
                        name=f"xs0{h}", bufs=1) for h in range(2)]
        xsl = [None] + [xp.tile([128, KT * ICW], bf16, tag=f"xs{c}",
                                name=f"xs{c}", bufs=1)
                        for c in range(1, IC)]
        wA = wp.tile([128, KT * 256], bf16, tag="wA", name="wA", bufs=1)
        wB = wp.tile([128, KT * 512], bf16, tag="wB", name="wB", bufs=1)
        for h in range(2):
            nc.sync.dma_start(
                xsl0[h][:],
                xT_d[:, h * (KT // 2) * ICW:(h + 1) * (KT // 2) * ICW])
        nc.sync.dma_start(wA[:], wqkA_d[:, :])
        nc.sync.dma_start(wB[:], wqkB_d[:, :])
        for c in range(1, IC):
            nc.sync.dma_start(xsl[c][:],
                              xT_d[:, c * KT * ICW:(c + 1) * KT * ICW])

        def xs(k, ic):
            if ic == 0:
                return xsl0[k // 4][:, (k % 4) * ICW:(k % 4 + 1) * ICW]
            return xsl[ic][:, k * ICW:(k + 1) * ICW]

        def wsrc(nt, k):
            """stationary slice for qkv group nt, contraction tile k."""
            if nt < 2:
                return wA[:, k * 256 + nt * 128:k * 256 + (nt + 1) * 128]
            return wB[:, k * 512 + (nt - 2) * 128:k * 512 + (nt - 1) * 128]
        wo2 = cst.tile([128, 2 * DIM], bf16, tag="wo2", name="wo2")
        nc.sync.dma_start(wo2[:], wout_d[:, :])
        wo = [wo2[:, d * DIM:(d + 1) * DIM] for d in range(2)]

        # ---- persistent activation tiles ----
        mu_bc = bcp.tile([128, N], f32, tag="mu")
        nrstd_bc = bcp.tile([128, N], bf16, tag="nrstd")
        q01 = qkp.tile([128, N], bf16, tag="q01")
        q23 = qkp.tile([128, N], bf16, tag="q23")
        k01 = qkp.tile([128, N], bf16, tag="k01")
        k23 = qkp.tile([128, N], bf16, tag="k23")
        vT = [vtp.tile([128, N], bf16, tag=f"vt{i}", name=f"vt{i}")
              for i in range(2)]
        qkv_dst = [q01, k01, q23, k23, vT[0], vT[1]]
        vaug = [[None] * JT for _ in range(2)]

        # ---- demand-driven emission ----
        emitted = set()

        def ensure(key, fn, *args):
            if key not in emitted:
                emitted.add(key)
                fn(*args)

        def stats_emit(ic):
            """LN row stats for one i-chunk: mu_bc, nrstd_bc columns."""
            isl = slice(ic * ICW, (ic + 1) * ICW)
            sum_ps = psum.tile([128, ICW], f32, tag="g", name="sum_ps")
            sq_ps = psum.tile([128, ICW], f32, tag="g", name="sq_ps")
            for k in range(KT):
                xk = xs(k, ic)
                x2 = scp.tile([128, ICW], bf16, tag="x2", bufs=8, name="x2")
                # ic0 is the head critical path: alternate engines there so
                # the x^2 stream finishes in half the serial time; later
                # chunks keep DVE free for the qkv epilogues.
                if GP_X2 and (ic > 0 or k % 2 == 1):
                    nc.gpsimd.tensor_mul(x2[:], xk, xk)
                else:
                    nc.vector.tensor_mul(x2[:], xk, xk)
                nc.tensor.matmul(sum_ps[:], ones[:], xk,
                                 start=(k == 0), stop=(k == KT - 1))
                nc.tensor.matmul(sq_ps[:], ones[:], x2[:],
                                 start=(k == 0), stop=(k == KT - 1))
            nc.vector.tensor_scalar_mul(mu_bc[:, isl], sum_ps[:], 1.0 / DIM)
            msq = scp.tile([128, ICW], f32, tag="msq", bufs=1, name="msq")
            nc.vector.tensor_scalar(msq[:], sq_ps[:], 1.0 / DIM, EPS,
                                    op0=ALU.mult, op1=ALU.add)
            mu2 = scp.tile([128, ICW], f32, tag="mu2", bufs=1, name="mu2")
            nc.vector.tensor_mul(mu2[:], mu_bc[:, isl], mu_bc[:, isl])
            var = scp.tile([128, ICW], f32, tag="var", bufs=1, name="var")
            nc.vector.tensor_sub(var[:], msq[:], mu2[:])
            if ic == 0:
                # head critical path: short ACT chain (Ln+Exp table loads
                # land before the attention exp stream starts)
                lnv = scp.tile([128, ICW], f32, tag="lnv", bufs=1, name="lnv")
                nc.scalar.activation(lnv[:], var[:], AF.Ln)
                rstd = scp.tile([128, ICW], f32, tag="rstd", bufs=1,
                                name="rstd")
                nc.scalar.activation(rstd[:], lnv[:], AF.Exp, scale=-0.5)
                nc.vector.tensor_scalar_mul(nrstd_bc[:, isl], rstd[:], -1.0)
                return
            # streamed stats: -1/sqrt(var) via bit-trick seed + 2 Newton
            # steps, all on DVE (no ACT table switches mid exp-stream)
            y0 = scp.tile([128, ICW], f32, tag="y0", bufs=1, name="y0")
            half_i = y0.bitcast(mybir.dt.int32)
            nc.vector.tensor_scalar(half_i[:], var.bitcast(mybir.dt.int32)[:],
                                    1, None, op0=ALU.arith_shift_right)
            nc.vector.scalar_tensor_tensor(half_i[:], magic_t[:], 1,
                                           half_i[:], op0=ALU.bypass,
                                           op1=ALU.subtract)
            t1 = scp.tile([128, ICW], f32, tag="t1", bufs=1, name="t1")
            nc.vector.tensor_mul(t1[:], y0[:], y0[:])
            nc.vector.tensor_mul(t1[:], t1[:], var[:])
            nc.vector.tensor_scalar(t1[:], t1[:], -0.5, 1.5,
                                    op0=ALU.mult, op1=ALU.add)
            nc.vector.tensor_mul(y0[:], y0[:], t1[:])
            nc.vector.tensor_mul(t1[:], y0[:], y0[:])
            nc.vector.tensor_mul(t1[:], t1[:], var[:])
            nc.vector.tensor_scalar(t1[:], t1[:], 0.5, 1.5,
                                    op0=ALU.mult, op1=ALU.subtract)
            nc.vector.tensor_mul(nrstd_bc[:, isl], y0[:], t1[:])

        def qkv_emit(nt, ic):
            ensure(("st", ic), stats_emit, ic)
            isl = slice(ic * ICW, (ic + 1) * ICW)
            ps = psum.tile([128, ICW], f32, tag="g", name="qkv_ps")
            for k in range(KT):
                nc.tensor.matmul(ps[:], wsrc(nt, k), xs(k, ic),
                                 start=(k == 0), stop=(k == KT - 1))
            # (mu*wsum - raw) * (-rstd) [+ wb]
            tmp = scp.tile([128, ICW], bf16, tag="fix", bufs=3, name="fix")
            nc.vector.scalar_tensor_tensor(
                tmp[:], mu_bc[:, isl], wsum_t[:, nt:nt + 1], ps[:],
                op0=ALU.mult, op1=ALU.subtract)
            dst = qkv_dst[nt][:, isl]
            nc.vector.tensor_mul(dst, tmp[:], nrstd_bc[:, isl])
            if has_wb:
                nc.vector.tensor_scalar_add(dst, dst, wb_t[:, nt:nt + 1])

        def tp_emit(d, j):
            """v^T -> v_aug[d][j]: [v_h | 1] blocks for the two local heads."""
            ensure(("g", 4 + d, j // 4), qkv_emit, 4 + d, j // 4)
            va = vaugp.tile([128, 2 * 65], bf16, tag=f"vaug{d}",
                            name=f"vaug{d}_{j}", bufs=JT)
            vaug[d][j] = va
            nc.vector.memset(va[:, 64:2 * 65:65], 1.0)
            tp = psum.tile([128, 128], bf16, tag="g", name="tp")
            nc.tensor.transpose(tp[:], vT[d][:, j * 128:(j + 1) * 128],
                                ident[:])
            nc.vector.tensor_copy(va[:, 0:64], tp[:, 0:64])
            nc.vector.tensor_copy(va[:, 65:129], tp[:, 64:128])

        def ensure_qkv(nt, ic):
            ensure(("g", nt, ic), qkv_emit, nt, ic)

        def ensure_tp(d, j):
            ensure(("tp", d, j), tp_emit, d, j)

        o_norm = [[onp.tile([128, ICW], bf16, tag="onorm",
                            name=f"onorm{p}_{i}") for i in range(IC)]
                  for p in range(2)]

        y0 = [onp.tile([128, ICW], f32, tag="y0", name=f"y0_{mt}")
              for mt in range(MT)]

        def outproj_emit(ic, mt):
            """One 128-row tile of the output projection for chunk ic."""
            isl = slice(ic * ICW, (ic + 1) * ICW)
            msl = slice(mt * 128, (mt + 1) * 128)
            pps = psum.tile([128, ICW], f32, tag="g", name="pj_ps")
            for d in range(2):
                nc.tensor.matmul(pps[:], wo[d][:, msl], o_norm[d][ic][:],
                                 start=(d == 0), stop=(d == 1))
            ot = otp.tile([128, ICW], bf16, tag="ot", name="ot")
            nc.vector.tensor_copy(ot[:], pps[:])
            nc.sync.dma_start(out_d[msl, isl], ot[:])

        def outproj0_emit(ic, mt):
            """d=0 half of the last chunk's projection (runs during the
            final attention loop; only needs pair 0's normalized output)."""
            msl = slice(mt * 128, (mt + 1) * 128)
            pps = psum.tile([128, ICW], f32, tag="g", name="pj0_ps")
            nc.tensor.matmul(pps[:], wo[0][:, msl], o_norm[0][ic][:])
            nc.vector.tensor_copy(y0[mt][:], pps[:])

        def outproj1_emit(ic, mt):
            """d=1 half + combine with the stashed d=0 half, then store."""
            isl = slice(ic * ICW, (ic + 1) * ICW)
            msl = slice(mt * 128, (mt + 1) * 128)
            pps = psum.tile([128, ICW], f32, tag="g", name="pj1_ps")
            nc.tensor.matmul(pps[:], wo[1][:, msl], o_norm[1][ic][:])
            ot = otp.tile([128, ICW], bf16, tag="ot", name="ot")
            nc.vector.tensor_add(ot[:], pps[:], y0[mt][:])
            nc.sync.dma_start(out_d[msl, isl], ot[:])

        # drip: spread deferred work (stats, k/v groups, v transposes, the
        # previous chunk's projection, the next chunk's q groups) into the
        # attention loop so it overlaps the exp stream with a long lead.
        drip = deque()

        def drip_one():
            while drip:
                key = drip.popleft()
                if key[0] == "op":
                    outproj_emit(key[1], key[2])
                elif key[0] == "op0":
                    outproj0_emit(key[1], key[2])
                elif key in emitted:
                    continue
                elif key[0] == "st":
                    ensure(key, stats_emit, key[1])
                elif key[0] == "tp":
                    ensure_tp(key[1], key[2])
                else:
                    ensure_qkv(key[1], key[2])
                return

        # ---- head: minimum work before the exp stream can start ----
        ensure_qkv(0, 0)          # q01 first chunk (pulls stats(0))
        ensure_qkv(1, 0)          # k01 first j-blocks

        # seed the drip with everything (ic0, pair0) will demand, in
        # deadline order, so the DVE epilogues run well before the PE
        # needs their results.
        for icc in range(1, IC):
            drip.append(("st", icc))
            drip.append(("g", 1, icc))
        for jj in range(IC):
            drip.append(("g", 4, jj))
            for j in range(4 * jj, 4 * jj + 4):
                drip.append(("tp", 0, j))

        # ---- attention (i-chunk-major) + dripped output projection ----
        qt_pair = [q01, q23]
        kt_pair = [k01, k23]
        for ic in range(IC):
            for pair in range(2):
                qt = qt_pair[pair]
                kt = kt_pair[pair]
                isl = slice(ic * ICW, (ic + 1) * ICW)
                ensure_qkv(2 * pair, ic)
                if ic == 0 and pair == 1:
                    for jj in range(1, IC):
                        drip.append(("g", 3, jj))
                    for jj in range(IC):
                        drip.append(("g", 5, jj))
                        for j in range(4 * jj, 4 * jj + 4):
                            drip.append(("tp", 1, j))
                if ic + 1 < IC:
                    drip.append(("g", 2 * pair, ic + 1))
                if ic == IC - 1 and pair == 1:
                    for mt in range(MT):
                        drip.append(("op0", ic, mt))
                o_ps = [psum.tile([128, ICW], f32, tag="o", name="o_ps")
                        for _ in range(2)]
                for j in range(JT):
                    jsl = slice(j * 128, (j + 1) * 128)
                    ensure_qkv(2 * pair + 1, j // 4)
                    s_ps = psum.tile([128, 2 * ICW], f32, tag="s",
                                     name="s_ps")
                    e_t = ep.tile([128, 2 * ICW], bf16, tag="e", name="e_t")
                    for hh in range(2):
                        psl = slice(hh * 64, (hh + 1) * 64)
                        nc.tensor.matmul(s_ps[:, hh * ICW:(hh + 1) * ICW],
                                         kt[psl, jsl], qt[psl, isl])
                    # no bias: exp(s) vs exp(s-4) cancels in the softmax
                    # normalization, and s ~ N(0,1) cannot overflow fp32
                    nc.scalar.activation(e_t[:], s_ps[:], AF.Exp)
                    if ic == 0:
                        drip_one()
                    ensure_tp(pair, j)
                    for hh in range(2):
                        nc.tensor.matmul(
                            o_ps[hh][0:65, :],
                            vaug[pair][j][:, 65 * hh:65 * hh + 65],
                            e_t[:, hh * ICW:(hh + 1) * ICW],
                            start=(j == 0), stop=(j == JT - 1))
                    # ic0 has a large emission backlog (2 pops per j);
                    # later chunks spread their few items across the loop
                    # so the exp stream stays the pacer nowhere.
                    if ic == 0 or j % 2 == 1:
                        drip_one()
                # normalize: O[d,i] / l_i with the O rows read from the PV
                # PSUM in place (no ACT copies). Only the l rows go through
                # SBUF: DVE copy (same partition), DMA down to partition 0,
                # reciprocal, then gpsimd broadcast across 64 partitions.
                lsb = smp.tile([66, 2 * ICW], f32, tag="lsb", name="lsb")
                lrow = [smp.tile([1, ICW], f32, tag=f"lrow{h}",
                                 name=f"lrow{h}") for h in range(2)]
                linv = [smp.tile([1, ICW], f32, tag=f"linv{h}",
                                 name=f"linv{h}") for h in range(2)]
                lbc = [smp.tile([64, ICW], f32, tag=f"lbc{h}",
                                name=f"lbc{h}") for h in range(2)]
                for hh in range(2):
                    csl = slice(hh * ICW, (hh + 1) * ICW)
                    nc.vector.tensor_copy(lsb[64:65, csl],
                                          o_ps[hh][64:65, :])
                    nc.sync.dma_start(lrow[hh][:], lsb[64:65, csl])
                    nc.vector.reciprocal_approx_fast(linv[hh][:], lrow[hh][:])
                    nc.gpsimd.partition_broadcast(lbc[hh][:, :], linv[hh][:])
                nc.vector.tensor_mul(o_norm[pair][ic][0:64, :],
                                     o_ps[0][0:64, :], lbc[0][:, :])
                ob = smp.tile([64, ICW], bf16, tag="ob", name="ob")
                nc.vector.tensor_mul(ob[:], o_ps[1][0:64, :], lbc[1][:, :])
                nc.sync.dma_start(o_norm[pair][ic][64:128, :], ob[:])
                if pair == 1:
                    if ic == IC - 1:
                        # tail: only the d=1 halves + combine remain
                        while drip:
                            drip_one()
                        for mt in range(MT):
                            outproj1_emit(ic, mt)
                    else:
                        # queue this chunk's projection into the next
                        # chunk's attention loop
                        for mt in range(MT):
                            drip.append(("op", ic, mt))
        while drip:
            drip_one()

    nc.compile()
    return nc


def _get_compiled(has_wb):
    key = bool(has_wb)
    if key not in _COMPILED:
        _COMPILED[key] = _build(key)
    return _COMPILED[key]


def _shard_inputs(x, ln_gamma, ln_beta, w_qkv, w_out):
    """Build per-core input maps (all host-side layout work, no math on x)."""
    import ml_dtypes
    bf = ml_dtypes.bfloat16

    x = np.ascontiguousarray(np.asarray(x, np.float32))
    g = np.asarray(ln_gamma, np.float32)
    be = np.asarray(ln_beta, np.float32)
    wq = np.asarray(w_qkv, np.float32)
    wo = np.asarray(w_out, np.float32)

    scale = DIM_HEAD ** (-0.5)
    wq_g = wq * g[:, None]            # gamma folded
    wq_g[:, :INNER] *= scale          # attention scale folded into W_q
    wb_full = be @ wq                 # beta contribution
    wb_full[:INNER] *= scale

    in_maps = []
    for c in range(N_CORES):
        b = c // HEADS_PER_CORE
        grp = c % HEADS_PER_CORE
        heads = [HEADS_PER_CORE * grp + t for t in range(HEADS_PER_CORE)]
        # column order: [q01, k01, q23, k23, v01, v23] pair-tiles
        cols = []
        for which, pair in [(0, 0), (1, 0), (0, 1), (1, 1), (2, 0), (2, 1)]:
            for h in (heads[2 * pair], heads[2 * pair + 1]):
                lo = which * INNER + h * DIM_HEAD
                cols.append(np.arange(lo, lo + DIM_HEAD))
        cols = np.concatenate(cols)
        wqkv_c = np.ascontiguousarray(wq_g[:, cols])
        # bf16-round the weights before computing wsum so the LN-fold
        # correction matches what the device matmul actually sums.
        wqkv_bf = wqkv_c.astype(bf)
        wsum_c = wqkv_bf.astype(np.float32).sum(axis=0)
        wb_c = wb_full[cols]
        rows = np.concatenate([np.arange(h * DIM_HEAD, (h + 1) * DIM_HEAD)
                               for h in heads])
        wout_c = np.ascontiguousarray(wo[rows, :])
        # pack everything to 128 partitions x contiguous slabs (single DMAs)
        xT = x[b].T.astype(bf)                       # [DIM, N]
        x_pk = xT.reshape(KT, 128, IC, ICW).transpose(1, 2, 0, 3)
        w_k = wqkv_bf.reshape(KT, 128, NQKV)
        wA_pk = w_k[:, :, 0:256].transpose(1, 0, 2)
        wB_pk = w_k[:, :, 256:NQKV].transpose(1, 0, 2)
        wo_pk = wout_c.astype(bf).reshape(2, 128, DIM).transpose(1, 0, 2)
        in_maps.append({
            "xT": np.ascontiguousarray(x_pk.reshape(128, IC * KT * ICW)),
            "wqkA": np.ascontiguousarray(wA_pk.reshape(128, KT * 256)),
            "wqkB": np.ascontiguousarray(wB_pk.reshape(128, KT * 512)),
            "wout": np.ascontiguousarray(wo_pk.reshape(128, 2 * DIM)),
            "wsum": np.ascontiguousarray(
                wsum_c.reshape(NT, 128).T.astype(np.float32)),
            "wb": np.ascontiguousarray(
                wb_c.reshape(NT, 128).T.astype(np.float32)),
        })
    return in_maps


def _run(inputs, trace=False):
    from concourse.bass_utils import run_bass_kernel_spmd

    in_maps = _shard_inputs(inputs["x"], inputs["ln_gamma"],
                            inputs["ln_beta"], inputs["w_qkv"],
                            inputs["w_out"])
    has_wb = bool(np.max(np.abs(in_maps[0]["wb"])) > 0)
    nc = _get_compiled(has_wb)
    res = run_bass_kernel_spmd(nc, in_maps, core_ids=list(range(N_CORES)),
                               trace=trace)
    b_out = np.asarray(inputs["b_out"], np.float32)
    outs = []
    for b in range(B):
        acc = np.zeros((DIM, N), np.float32)
        for grp in range(HEADS_PER_CORE):
            acc += res.results[b * HEADS_PER_CORE + grp]["outT"].astype(
                np.float32)
        outs.append(acc.T + b_out)
    out = np.stack(outs).astype(np.float32)
    return out, res


def kernel(**inputs):
    out, _ = _run(inputs, trace=False)
    return out
